# revision 20
# baseline (speedup 1.0000x reference)
"""Trainium2 Bass kernel for nn_ActorNet (MLP -> 2-layer LSTM(543) -> heads).

Strategy: the `done` flags (p=0.5) cut every batch lane into independent
segments (mean length 2, max ~20).  The host splits the (T=512, B=128)
problem into ~33k segments, bin-packs them into 4096 virtual lanes x Tp
(~16-18) steps, and the device runs the SAME recurrence with 512 lanes
per core per step -> full TensorE utilization, pure data parallel, no
collectives.  Outputs are scattered back on the host.

Device layout: everything transposed (features on partitions, lanes on
the free dim).  Gate weights are column-reordered hidden-tile-major so
i/f/g/o tiles are 128-aligned; biases are baked into the recurrent
matmul via a constant-1 row in the h-side state.  Matmuls run in bf16
(fp32 PSUM accumulation); exp/tanh output heads are deferred to a
post-pass (ACT table-set switch).
"""
import os
import sys
from contextlib import ExitStack

sys.path.insert(0, "/opt/trn_rl_repo")
import numpy as np
import ml_dtypes

BF16 = ml_dtypes.bfloat16

T, B = 512, 128
SD, AD, ND = 128, 30, 512
CORE = ND + AD + 1            # 543
NCORES = 8
LC = 512                      # lanes per core
LANES = NCORES * LC           # 4096
P = 128
TAIL = CORE - 4 * P           # 31
MW = 16 * P + (4 * TAIL + 3)  # 2175 reordered gate cols (16 full tiles + mixed 127)
KH_X = [128, 128, 128, 128, TAIL]       # x-side K-tile heights (543)
KH_H = [128, 128, 128, 128, TAIL + 2]   # h-side (543 + zero row + ones/bias row @p32)

LAST_EXEC_NS = None
_CACHE = {}


# ---------------------------------------------------------------- packing
def _segments(done):
    """done [T,B] bool -> list of (length, b, t0), state resets AFTER a done slot."""
    segs = []
    for b in range(B):
        t0 = 0
        col = done[:, b]
        for t in range(T):
            if col[t]:
                segs.append((t - t0 + 1, b, t0))
                t0 = t + 1
        if t0 < T:
            segs.append((T - t0, b, t0))
    return segs


def _try_pack(segs, cap):
    """Best-fit-decreasing into LANES lanes of capacity cap. segs sorted desc."""
    buckets = [[] for _ in range(cap + 1)]
    buckets[cap] = list(range(LANES - 1, -1, -1))
    assign = [[] for _ in range(LANES)]
    for (l, b, t0) in segs:
        r = -1
        for rr in range(l, cap + 1):
            if buckets[rr]:
                r = rr
                break
        if r < 0:
            return None
        lane = buckets[r].pop()
        assign[lane].append((b, t0, l))
        buckets[r - l].append(lane)
    return assign


def _build_packing(done):
    segs = _segments(done)
    total = sum(s[0] for s in segs)
    assert total == T * B
    segs.sort(key=lambda s: -s[0])
    Tp = max(segs[0][0], (total + LANES - 1) // LANES)
    while True:
        assign = _try_pack(segs, Tp)
        if assign is not None:
            break
        Tp += 1
    src = np.full((LANES, Tp), -1, np.int64)
    mask = np.zeros((LANES, Tp), np.float32)
    for lane, lst in enumerate(assign):
        pos = 0
        for (b, t0, l) in lst:
            src[lane, pos:pos + l] = (np.arange(t0, t0 + l) * B + b)
            mask[lane, pos:pos + l - 1] = 1.0   # last slot of each segment -> 0
            pos += l
    return Tp, src, mask


def _pack_inputs(frame, reward, last_action, src, mask):
    ff = frame.reshape(T * B, SD)
    rf = reward.reshape(T * B)
    lf = last_action.reshape(T * B, AD)
    per_core = []
    for c in range(NCORES):
        s = src[c * LC:(c + 1) * LC]
        m = mask[c * LC:(c + 1) * LC]
        idx = s.T.reshape(-1)                   # tau-major columns
        ok = idx >= 0
        ii = np.where(ok, idx, 0)
        fT = np.ascontiguousarray(np.where(ok[None, :], ff[ii].T, 0.0)).astype(BF16)
        ex = np.zeros((31, idx.size), np.float32)
        ex[0] = np.where(ok, rf[ii], 0.0)
        ex[1:31] = np.where(ok[None, :], lf[ii].T, 0.0)
        mk = np.ascontiguousarray(
            np.broadcast_to(m.T.reshape(1, -1), (P, idx.size))).astype(BF16)
        per_core.append({"frameT": fT, "extrasT": ex.astype(BF16), "maskT": mk})
    return per_core


# ---------------------------------------------------------------- weights
def _gate_perm():
    perm = np.full(MW, -1, np.int64)
    pos = 0
    for j in range(4):
        for g in range(4):
            perm[pos:pos + P] = g * CORE + np.arange(j * P, (j + 1) * P)
            pos += P
    for g in range(4):                           # mixed tile: i31 _ f31 _ g31 _ o31
        perm[pos:pos + TAIL] = g * CORE + np.arange(4 * P, CORE)
        pos += TAIL
        if g < 3:
            pos += 1
    assert pos == MW
    return perm


def _colvec(v):
    out = np.zeros((P, 1), np.float32)
    v = np.asarray(v).reshape(-1)
    out[:v.size, 0] = v
    return out


def _prep_weights(p):
    perm = _gate_perm()
    ok = perm >= 0

    def reorder(wt):                             # wt [in, 4*CORE] -> [in, MW]
        out = np.zeros((wt.shape[0], MW), np.float32)
        out[:, ok] = wt[:, perm[ok]]
        return out

    w = {}
    for l in range(2):
        wi, wh = p[f"Wih{l}"], p[f"Whh{l}"]
        bias = np.zeros(MW, np.float32)
        bias[ok] = (p[f"bih{l}"] + p[f"bhh{l}"])[perm[ok]]
        wixr = reorder(wi.T)                     # [543, MW]
        whhr = reorder(wh.T)
        w[f"wix{l}"] = np.ascontiguousarray(wixr[0:512]).astype(BF16)
        w[f"whh{l}"] = np.ascontiguousarray(whhr[0:512]).astype(BF16)
        # combined tail K-tile: [x-tail 0:31 | 0 | bias@32 | 0 | h-tail 64:95 | 0]
        wct = np.zeros((P, MW), np.float32)
        wct[0:TAIL] = wixr[512:CORE]
        wct[32] = bias
        wct[64:64 + TAIL] = whhr[512:CORE]
        w[f"wct{l}"] = wct.astype(BF16)

    w["ws0t"] = np.ascontiguousarray(p["Ws0"].T).astype(BF16)       # [128,512]
    w["bs0t"] = np.ascontiguousarray(p["bs0"].reshape(4, P).T, np.float32)
    w["ws1t"] = np.ascontiguousarray(p["Ws1"].T).astype(BF16)       # [512,512]
    w["bs1t"] = np.ascontiguousarray(p["bs1"].reshape(4, P).T, np.float32)

    wp0t = np.zeros((640, 512), np.float32)
    wp0t[0:CORE] = p["Wp0"].T
    w["wp0t"] = wp0t.astype(BF16)
    w["bp0t"] = np.ascontiguousarray(p["bp0"].reshape(4, P).T, np.float32)
    w["wp1t"] = np.ascontiguousarray(p["Wp1"].T).astype(BF16)       # [512,30]
    w["bp1t"] = _colvec(p["bp1"])
    wlst = np.zeros((640, 30), np.float32)
    wlst[0:CORE] = p["Wls"].T
    w["wlst"] = wlst.astype(BF16)
    w["blst"] = _colvec(p["bls"])
    wb0t = np.zeros((640, 32), np.float32)
    wb0t[0:CORE] = p["Wb0"].T
    w["wb0t"] = wb0t.astype(BF16)
    w["bb0t"] = _colvec(p["bb0"])
    w["wb1t"] = np.ascontiguousarray(p["Wb1"].T).astype(BF16)       # [32,1]
    w["bb1t"] = _colvec(p["bb1"])
    return w


# ---------------------------------------------------------------- numpy emulator
def _emu_core(ins, p):
    """Numpy mirror of the device math (original weight order) for validation."""
    sig = lambda x: 1.0 / (1.0 + np.exp(-x))
    fT = np.asarray(ins["frameT"], np.float32)
    ex = np.asarray(ins["extrasT"], np.float32)
    mk = np.asarray(ins["maskT"], np.float32)
    TOT = fT.shape[1]
    Tp = TOT // LC
    x1 = np.maximum(p["Ws0"] @ fT + p["bs0"][:, None], 0)
    x2 = np.maximum(p["Ws1"] @ x1 + p["bs1"][:, None], 0)
    X = np.concatenate([x2, np.clip(ex[0], -1, 1)[None], ex[1:31]], 0)  # [543,TOT]
    out = np.zeros((97, TOT), np.float32)
    h1 = np.zeros((CORE, LC), np.float32)
    c1 = np.zeros_like(h1)
    h2 = np.zeros_like(h1)
    c2 = np.zeros_like(h1)
    b1 = (p["bih0"] + p["bhh0"])[:, None]
    b2 = (p["bih1"] + p["bhh1"])[:, None]
    hsw = lambda x: x * np.clip(x + 3.0, 0, 6.0) / 6.0
    for tau in range(Tp):
        sl = slice(tau * LC, (tau + 1) * LC)
        m = mk[0:1, sl]
        xs = X[:, sl]
        g = p["Wih0"] @ xs + p["Whh0"] @ h1 + b1
        i, f, gg, o = np.split(g, 4, 0)
        c1n = sig(f) * c1 + sig(i) * np.tanh(gg)
        h1r = sig(o) * np.tanh(c1n)
        h1, c1 = h1r * m, c1n * m
        g = p["Wih1"] @ h1r + p["Whh1"] @ h2 + b2
        i, f, gg, o = np.split(g, 4, 0)
        c2n = sig(f) * c2 + sig(i) * np.tanh(gg)
        y = sig(o) * np.tanh(c2n)
        h2, c2 = y * m, c2n * m
        ry = np.maximum(y, 0)
        z1 = p["Wp0"] @ ry + p["bp0"][:, None]
        am = p["Wp1"] @ hsw(z1) + p["bp1"][:, None]
        ls = np.clip(p["Wls"] @ hsw(y) + p["bls"][:, None], -20.0, 2.0)
        rb = np.maximum(p["Wb0"] @ ry + p["bb0"][:, None], 0)
        bl = p["Wb1"] @ rb + p["bb1"][:, None]
        out[0:30, sl] = am
        out[32:62, sl] = np.exp(ls)
        out[64:94, sl] = np.tanh(am)
        out[96, sl] = bl[0]
    return out


# ---------------------------------------------------------------- device kernel
def _build(Tp):
    import concourse.bass as bass
    import concourse.mybir as mybir
    from concourse import bacc
    from concourse.tile import TileContext

    f32 = mybir.dt.float32
    bf = mybir.dt.bfloat16
    AF = mybir.ActivationFunctionType
    AL = mybir.AluOpType
    n = LC
    TOT = Tp * n

    nc = bacc.Bacc(None)
    dp = lambda name, shape, dt, out=False: nc.declare_dram_parameter(
        name, list(shape), dt, isOutput=out)
    frameT = dp("frameT", (P, TOT), bf)
    extrasT = dp("extrasT", (31, TOT), bf)
    maskT = dp("maskT", (P, TOT), bf)
    ws0t = dp("ws0t", (P, 512), bf)
    bs0t = dp("bs0t", (P, 4), f32)
    ws1t = dp("ws1t", (512, 512), bf)
    bs1t = dp("bs1t", (P, 4), f32)
    wix = [dp("wix0", (512, MW), bf), dp("wix1", (512, MW), bf)]
    whh = [dp("whh0", (512, MW), bf), dp("whh1", (512, MW), bf)]
    wct = [dp("wct0", (P, MW), bf), dp("wct1", (P, MW), bf)]
    wp0t = dp("wp0t", (640, 512), bf)
    bp0t = dp("bp0t", (P, 4), f32)
    wp1t = dp("wp1t", (512, 30), bf)
    bp1t = dp("bp1t", (P, 1), f32)
    wlst = dp("wlst", (640, 30), bf)
    blst = dp("blst", (P, 1), f32)
    wb0t = dp("wb0t", (640, 32), bf)
    bb0t = dp("bb0t", (P, 1), f32)
    wb1t = dp("wb1t", (32, 1), bf)
    bb1t = dp("bb1t", (P, 1), f32)
    out97 = dp("out97", (97, TOT), f32, out=True)

    with ExitStack() as ctx:
        tc = ctx.enter_context(TileContext(nc))
        wpool = ctx.enter_context(tc.tile_pool(name="wpool", bufs=2))
        conp = ctx.enter_context(tc.tile_pool(name="conp", bufs=1))
        stp = ctx.enter_context(tc.tile_pool(name="stp", bufs=1))
        gat = ctx.enter_context(tc.tile_pool(name="gat", bufs=1))
        scr2 = ctx.enter_context(tc.tile_pool(name="scr2", bufs=2))
        scr1 = ctx.enter_context(tc.tile_pool(name="scr1", bufs=1))
        iop = ctx.enter_context(tc.tile_pool(name="iop", bufs=3))
        psp = ctx.enter_context(tc.tile_pool(name="psp", bufs=8, space="PSUM"))
        drp = ctx.enter_context(tc.tile_pool(name="drp", bufs=1, space="DRAM"))

        def dma(dst, src):
            nc.sync.dma_start(out=dst, in_=src)

        # ---- small persistent weights
        WS0 = conp.tile([P, 512], bf, tag="WS0", name="WS0")
        dma(WS0[:], ws0t[:])
        WS1 = conp.tile([P, 4 * 512], bf, tag="WS1", name="WS1")
        for k in range(4):
            dma(WS1[:, k * 512:(k + 1) * 512], ws1t[k * P:(k + 1) * P, :])
        WP0 = conp.tile([P, 5 * 512], bf, tag="WP0", name="WP0")
        for k in range(5):
            dma(WP0[:, k * 512:(k + 1) * 512], wp0t[k * P:(k + 1) * P, :])
        WP1 = conp.tile([P, 4 * 30], bf, tag="WP1", name="WP1")
        for k in range(4):
            dma(WP1[:, k * 30:(k + 1) * 30], wp1t[k * P:(k + 1) * P, :])
        WLS = conp.tile([P, 5 * 30], bf, tag="WLS", name="WLS")
        for k in range(5):
            dma(WLS[:, k * 30:(k + 1) * 30], wlst[k * P:(k + 1) * P, :])
        WB0 = conp.tile([P, 5 * 32], bf, tag="WB0", name="WB0")
        for k in range(5):
            dma(WB0[:, k * 32:(k + 1) * 32], wb0t[k * P:(k + 1) * P, :])
        WB1 = conp.tile([32, 1], bf, tag="WB1", name="WB1")
        dma(WB1[:], wb1t[:])
        BS0 = conp.tile([P, 4], f32, tag="BS0", name="BS0")
        dma(BS0[:], bs0t[:])
        BS1 = conp.tile([P, 4], f32, tag="BS1", name="BS1")
        dma(BS1[:], bs1t[:])
        BP0 = conp.tile([P, 4], f32, tag="BP0", name="BP0")
        dma(BP0[:], bp0t[:])
        BP1 = conp.tile([P, 1], f32, tag="BP1", name="BP1")
        dma(BP1[:], bp1t[:])
        BLS = conp.tile([P, 1], f32, tag="BLS", name="BLS")
        dma(BLS[:], blst[:])
        BB0 = conp.tile([P, 1], f32, tag="BB0", name="BB0")
        dma(BB0[:], bb0t[:])
        BB1 = conp.tile([P, 1], f32, tag="BB1", name="BB1")
        dma(BB1[:], bb1t[:])
        ZB = conp.tile([P, 1], f32, tag="ZB", name="ZB")
        nc.vector.memset(ZB[:], 0.0)
        ZBH = conp.tile([P, 1], bf, tag="ZBH", name="ZBH")
        nc.vector.memset(ZBH[:], 0.0)

        def load_big(wx, wh, wc):
            W = wpool.tile([P, 9 * MW], bf, tag="bigw", name="bigw")
            for k in range(4):
                dma(W[:, k * MW:(k + 1) * MW], wx[k * P:(k + 1) * P, :])
            for k in range(4):
                dma(W[:, (4 + k) * MW:(5 + k) * MW], wh[k * P:(k + 1) * P, :])
            dma(W[:, 8 * MW:9 * MW], wc[:])
            return W

        def new_state(nm):
            Hs = stp.tile([P, 4 * n], bf, tag="H" + nm, name="H" + nm)
            Cs = stp.tile([P, 5 * n], bf, tag="C" + nm, name="C" + nm)
            Ct = stp.tile([P, n], bf, tag="T" + nm, name="T" + nm)
            nc.vector.memset(Hs[:], 0.0)
            nc.vector.memset(Cs[:], 0.0)
            nc.vector.memset(Ct[:], 0.0)
            nc.vector.memset(Ct[32:33, :], 1.0)            # bias ones row
            return Hs, Cs, Ct

        FUNCS = [AF.Sigmoid, AF.Sigmoid, AF.Tanh, AF.Sigmoid]

        def cell(WL, xs, Hst, Cst, CTt, MK, tail_dst=None):
            """One LSTM cell step. xs(k) -> x-side rhs K-tile AP (bf16), k=0..3.
            Returns the unmasked h' tile [P, 5n] bf16 (valid rows per tile)."""
            SI = gat.tile([P, 5 * n], bf, tag="SI", name="SI")
            SF = gat.tile([P, 5 * n], bf, tag="SF", name="SF")
            TG = gat.tile([P, 5 * n], bf, tag="TG", name="TG")
            SO = gat.tile([P, 5 * n], bf, tag="SO", name="SO", bufs=2)
            GB = [SI, SF, TG, SO]

            def mseq(out_ap, mcol, mwid):
                for k in range(4):
                    nc.tensor.matmul(
                        out_ap,
                        WL[:, k * MW + mcol:k * MW + mcol + mwid],
                        xs(k), start=(k == 0), stop=False)
                for k in range(4):
                    nc.tensor.matmul(
                        out_ap,
                        WL[:, (4 + k) * MW + mcol:(4 + k) * MW + mcol + mwid],
                        Hst[:, k * n:(k + 1) * n],
                        start=False, stop=False)
                nc.tensor.matmul(
                    out_ap,
                    WL[:, 8 * MW + mcol:8 * MW + mcol + mwid],
                    CTt[:], start=False, stop=True)

            def elem(j):
                # per-hidden-tile LSTM update; pipelines under later waves' MMs
                rh = P if j < 4 else TAIL
                cs = slice(j * n, (j + 1) * n)
                nc.vector.tensor_tensor(TG[0:rh, cs], SI[0:rh, cs], TG[0:rh, cs],
                                        AL.mult)                       # sig(i)*tanh(g)
                nc.vector.tensor_tensor(SF[0:rh, cs], SF[0:rh, cs], Cst[0:rh, cs],
                                        AL.mult)                       # sig(f)*c
                nc.vector.tensor_tensor(Cst[0:rh, cs], TG[0:rh, cs], SF[0:rh, cs],
                                        AL.add)                        # c'
                nc.scalar.activation(SI[0:rh, cs], Cst[0:rh, cs], AF.Tanh,
                                     bias=ZBH[0:rh])                   # tanh(c')
                nc.vector.tensor_tensor(SO[0:rh, cs], SO[0:rh, cs], SI[0:rh, cs],
                                        AL.mult)                       # h'
                nc.vector.tensor_tensor(Cst[0:rh, cs], Cst[0:rh, cs],
                                        MK[0:rh, :], AL.mult)

            def state_write(j):
                # deferred until all this cell's h-side matmuls have read Hst
                cs = slice(j * n, (j + 1) * n)
                if j < 4:
                    nc.vector.tensor_tensor(Hst[:, cs], SO[:, cs], MK[:, :],
                                            AL.mult)
                else:
                    nc.vector.tensor_tensor(CTt[64:64 + TAIL, :],
                                            SO[0:TAIL, cs], MK[0:TAIL, :],
                                            AL.mult)

            psm = psp.tile([P, n], f32, tag="ps", name="psm")
            mseq(psm[0:127, :], 16 * P, 127)
            for g in range(4):
                nc.scalar.activation(
                    GB[g][0:TAIL, 4 * n:5 * n], psm[32 * g:32 * g + TAIL, :],
                    FUNCS[g], bias=ZB[0:TAIL])
            elem(4)
            if tail_dst is not None:
                nc.vector.tensor_copy(tail_dst[0:TAIL, :], SO[0:TAIL, 4 * n:5 * n])
            for j in range(4):
                pss = [psp.tile([P, n], f32, tag="ps", name="ps") for _ in range(4)]
                for g in range(4):
                    mseq(pss[g][:], (4 * j + g) * P, P)
                for g in range(4):
                    nc.scalar.activation(
                        GB[g][:, j * n:(j + 1) * n], pss[g][:],
                        FUNCS[g], bias=ZB[:])
                elem(j)
            for j in range(5):
                state_write(j)
            return SO

        def stage1(tau, CT1):
            FR = iop.tile([P, n], bf, tag="FR", name="FR")
            dma(FR[:], frameT[:, tau * n:(tau + 1) * n])
            X = scr2.tile([P, 4 * n], bf, tag="s5a", name="X")
            dma(CT1[0:31, :], extrasT[:, tau * n:(tau + 1) * n])
            nc.vector.tensor_scalar(CT1[0:1, :], CT1[0:1, :],
                                    1.0, -1.0, AL.min, AL.max)
            X1 = scr1.tile([P, 4 * n], bf, tag="s4a", name="X1")
            for m in range(4):
                ps = psp.tile([P, n], f32, tag="ps", name="ps1")
                nc.tensor.matmul(ps[:], WS0[:, m * P:(m + 1) * P], FR[:],
                                 start=True, stop=True)
                nc.vector.tensor_scalar(X1[:, m * n:(m + 1) * n], ps[:],
                                        BS0[:, m:m + 1], 0.0, AL.add, AL.max)
            for m in range(4):
                ps = psp.tile([P, n], f32, tag="ps", name="ps2")
                for k in range(4):
                    nc.tensor.matmul(ps[:],
                                     WS1[:, k * 512 + m * P:k * 512 + (m + 1) * P],
                                     X1[:, k * n:(k + 1) * n],
                                     start=(k == 0), stop=(k == 3))
                nc.vector.tensor_scalar(X[:, m * n:(m + 1) * n], ps[:],
                                        BS1[:, m:m + 1], 0.0, AL.add, AL.max)
            return X

        def heads(tau, Y):
            OT = iop.tile([97, n], f32, tag="OT", name="OT")
            RY = scr1.tile([P, 5 * n], bf, tag="s5b", name="RY")
            T1 = scr1.tile([P, 5 * n], bf, tag="s5d", name="T1")
            HY = scr1.tile([P, 5 * n], bf, tag="s5c", name="HY")
            for j in range(5):
                rh = P if j < 4 else TAIL
                cs = slice(j * n, (j + 1) * n)
                nc.vector.tensor_scalar_max(RY[0:rh, cs], Y[0:rh, cs], 0.0)
                nc.vector.tensor_scalar(T1[0:rh, cs], Y[0:rh, cs], 3.0, 0.0,
                                        AL.add, AL.max)
                nc.vector.tensor_scalar_min(T1[0:rh, cs], T1[0:rh, cs], 6.0)
                nc.vector.scalar_tensor_tensor(HY[0:rh, cs], Y[0:rh, cs],
                                               1.0 / 6.0, T1[0:rh, cs],
                                               AL.mult, AL.mult)
            Z1 = scr1.tile([P, 4 * n], bf, tag="s4b", name="Z1")
            T2 = scr1.tile([P, 4 * n], bf, tag="s4c", name="T2")
            for m in range(4):
                ps = psp.tile([P, n], f32, tag="ps", name="ps3")
                for k in range(5):
                    nc.tensor.matmul(ps[:],
                                     WP0[0:KH_X[k],
                                         k * 512 + m * P:k * 512 + (m + 1) * P],
                                     RY[0:KH_X[k], k * n:(k + 1) * n],
                                     start=(k == 0), stop=(k == 4))
                cs = slice(m * n, (m + 1) * n)
                nc.vector.tensor_scalar_add(Z1[:, cs], ps[:], BP0[:, m:m + 1])
                nc.vector.tensor_scalar(T2[:, cs], Z1[:, cs], 3.0, 0.0,
                                        AL.add, AL.max)
                nc.vector.tensor_scalar_min(T2[:, cs], T2[:, cs], 6.0)
                nc.vector.scalar_tensor_tensor(Z1[:, cs], Z1[:, cs], 1.0 / 6.0,
                                               T2[:, cs], AL.mult, AL.mult)
            psA = psp.tile([P, n], f32, tag="ps", name="psA")
            for k in range(4):
                nc.tensor.matmul(psA[0:30, :], WP1[:, k * 30:(k + 1) * 30],
                                 Z1[:, k * n:(k + 1) * n],
                                 start=(k == 0), stop=(k == 3))
            nc.vector.tensor_scalar_add(OT[0:30, :], psA[0:30, :], BP1[0:30, :])
            nc.scalar.activation(OT[64:94, :], OT[0:30, :], AF.Tanh,
                                 bias=ZB[0:30])
            psB = psp.tile([P, n], f32, tag="ps", name="psB")
            for k in range(5):
                nc.tensor.matmul(psB[0:32, :], WB0[0:KH_X[k], k * 32:(k + 1) * 32],
                                 RY[0:KH_X[k], k * n:(k + 1) * n],
                                 start=(k == 0), stop=(k == 4))
            RB = scr1.tile([P, n], bf, tag="s1", name="RB")
            nc.vector.tensor_scalar(RB[0:32, :], psB[0:32, :],
                                    BB0[0:32, :], 0.0, AL.add, AL.max)
            psC = psp.tile([P, n], f32, tag="ps", name="psC")
            nc.tensor.matmul(psC[0:1, :], WB1[0:32, 0:1], RB[0:32, :],
                             start=True, stop=True)
            nc.vector.tensor_scalar_add(OT[96:97, :], psC[0:1, :], BB1[0:1, :])
            psL = psp.tile([P, n], f32, tag="ps", name="psL")
            for k in range(5):
                nc.tensor.matmul(psL[0:30, :], WLS[0:KH_X[k], k * 30:(k + 1) * 30],
                                 HY[0:KH_X[k], k * n:(k + 1) * n],
                                 start=(k == 0), stop=(k == 4))
            UL = scr1.tile([P, n], f32, tag="s1b", name="UL")
            nc.vector.tensor_scalar(UL[0:30, :], psL[0:30, :],
                                    BLS[0:30, :], 2.0, AL.add, AL.min)
            nc.vector.tensor_scalar_max(UL[0:30, :], UL[0:30, :], -20.0)
            # exp(u) = sig(u)/(1-sig(u)) -- stays in the sigmoid table set
            SG = scr1.tile([P, n], f32, tag="s1c", name="SG")
            nc.scalar.activation(SG[0:30, :], UL[0:30, :], AF.Sigmoid,
                                 bias=ZB[0:30])
            nc.vector.tensor_scalar(UL[0:30, :], SG[0:30, :], -1.0, 1.0,
                                    AL.mult, AL.add)                  # 1-s
            nc.vector.reciprocal(UL[0:30, :], UL[0:30, :])
            nc.vector.tensor_tensor(OT[32:62, :], SG[0:30, :], UL[0:30, :],
                                    AL.mult)
            dma(out97[:, tau * n:(tau + 1) * n], OT[0:97, :])

        # ---- single phase: stage-1 MLP + both LSTM layers + heads per step
        H1m, C1, CT1 = new_state("1")
        H2m, C2, CT2 = new_state("2")
        MK = iop.tile([P, n], bf, tag="MK", name="MK")
        dma(MK[:], maskT[:, 0:n])
        X = stage1(0, CT1)
        WL0 = load_big(wix[0], whh[0], wct[0])
        WL1 = load_big(wix[1], whh[1], wct[1])
        H1R = cell(WL0, lambda k: X[:, k * n:(k + 1) * n], H1m, C1, CT1, MK,
                   tail_dst=CT2)
        # software pipeline: cell1(tau+1) fills cell2(tau)'s tail gap,
        # heads(tau) fills cell1(tau+1)'s tail gap.
        for tau in range(Tp):
            if tau + 1 < Tp:
                X = stage1(tau + 1, CT1)
            Y = cell(WL1, lambda k: H1R[:, k * n:(k + 1) * n], H2m, C2, CT2, MK)
            if tau + 1 < Tp:
                MK = iop.tile([P, n], bf, tag="MK", name="MK")
                dma(MK[:], maskT[:, (tau + 1) * n:(tau + 2) * n])
                H1R = cell(WL0, lambda k: X[:, k * n:(k + 1) * n], H1m, C1, CT1,
                           MK, tail_dst=CT2)
            heads(tau, Y)

    nc.compile()
    return nc


# ---------------------------------------------------------------- entry point
def _unpack(outs, src):
    TB = T * B
    am = np.zeros((TB, AD), np.float32)
    std = np.zeros((TB, AD), np.float32)
    act = np.zeros((TB, AD), np.float32)
    bl = np.zeros(TB, np.float32)
    for c in range(NCORES):
        s = src[c * LC:(c + 1) * LC]
        idx = s.T.reshape(-1)
        ok = idx >= 0
        o = outs[c]
        am[idx[ok]] = o[0:30][:, ok].T
        std[idx[ok]] = o[32:62][:, ok].T
        act[idx[ok]] = o[64:94][:, ok].T
        bl[idx[ok]] = o[96, ok]
    pl = np.concatenate([am, std], -1).reshape(T, B, 2 * AD)
    return pl, bl.reshape(T, B), act.reshape(T, B, AD)


def _install_ntff_hook():
    """This image's antenv lacks axon_hooks; inject it so trace=True can
    drive NTFF profiling via the libaxon .so (same recipe as trn_boot)."""
    import types
    try:
        from antenv.axon_hooks import get_axon_ntff_profile_hook  # noqa
        return
    except ImportError:
        pass
    sys.path.insert(0, "/root/.axon_site")
    from trn_agent_boot.trn_boot import _ntff_profile_via_ctypes
    hook = _ntff_profile_via_ctypes("/opt/axon/libaxon_pjrt.so")
    mod = types.ModuleType("antenv.axon_hooks")
    mod._hook = hook
    mod.set_axon_ntff_profile_hook = lambda h: setattr(mod, "_hook", h)
    mod.get_axon_ntff_profile_hook = lambda: mod._hook
    import antenv
    antenv.axon_hooks = mod
    sys.modules["antenv.axon_hooks"] = mod
    import concourse.bass_utils as bu
    bu.upload_artifacts = lambda tmpdir: f"local:{tmpdir}"


def kernel(**inputs):
    global LAST_EXEC_NS
    p = {k: np.asarray(v) for k, v in inputs.items()}
    done = np.asarray(p["done"]).astype(bool)
    Tp, src, mask = _build_packing(done)
    per_core = _pack_inputs(np.asarray(p["frame"], np.float32),
                            np.asarray(p["reward"], np.float32),
                            np.asarray(p["last_action"], np.float32),
                            src, mask)
    if os.environ.get("KMODE", "hw") == "emu":
        outs = [_emu_core(ins, p) for ins in per_core]
        return _unpack(outs, src)

    w = _prep_weights(p)
    if Tp not in _CACHE:
        _CACHE[Tp] = _build(Tp)
    nc = _CACHE[Tp]
    in_maps = [{**w, **ins} for ins in per_core]
    from concourse.bass_utils import run_bass_kernel_spmd
    trace = bool(int(os.environ.get("KTRACE", "0")))
    if trace:
        _install_ntff_hook()
    res = run_bass_kernel_spmd(
        nc, in_maps, core_ids=list(range(NCORES)),
        trace=trace)
    LAST_EXEC_NS = res.exec_time_ns
    outs = [r["out97"] for r in res.results]
    return _unpack(outs, src)


# revision 21
# speedup vs baseline: 1.1967x; 1.1967x over previous
"""Trainium2 Bass kernel for nn_ActorNet (MLP -> 2-layer LSTM(543) -> heads).

Strategy: the `done` flags (p=0.5) cut every batch lane into independent
segments (mean length 2, max ~20).  The host splits the (T=512, B=128)
problem into ~33k segments, bin-packs them into 4096 virtual lanes x Tp
(~16-18) steps, and the device runs the SAME recurrence with 512 lanes
per core per step -> full TensorE utilization, pure data parallel, no
collectives.  Outputs are scattered back on the host.

Device layout: everything transposed (features on partitions, lanes on
the free dim).  Gate weights are column-reordered hidden-tile-major so
i/f/g/o tiles are 128-aligned; biases are baked into the recurrent
matmul via a constant-1 row in the h-side state.  Matmuls run in bf16
(fp32 PSUM accumulation); exp/tanh output heads are deferred to a
post-pass (ACT table-set switch).
"""
import os
import sys
from contextlib import ExitStack

sys.path.insert(0, "/opt/trn_rl_repo")
import numpy as np
import ml_dtypes

BF16 = ml_dtypes.bfloat16

T, B = 512, 128
SD, AD, ND = 128, 30, 512
CORE = ND + AD + 1            # 543
NCORES = 8
LC = 512                      # lanes per core
LANES = NCORES * LC           # 4096
P = 128
TAIL = CORE - 4 * P           # 31
MW = 16 * P + (4 * TAIL + 3)  # 2175 reordered gate cols (16 full tiles + mixed 127)
KH_X = [128, 128, 128, 128, TAIL]       # x-side K-tile heights (543)
KH_H = [128, 128, 128, 128, TAIL + 2]   # h-side (543 + zero row + ones/bias row @p32)

LAST_EXEC_NS = None
_CACHE = {}


# ---------------------------------------------------------------- packing
def _segments(done):
    """done [T,B] bool -> list of (length, b, t0), state resets AFTER a done slot."""
    segs = []
    for b in range(B):
        t0 = 0
        col = done[:, b]
        for t in range(T):
            if col[t]:
                segs.append((t - t0 + 1, b, t0))
                t0 = t + 1
        if t0 < T:
            segs.append((T - t0, b, t0))
    return segs


def _try_pack(segs, cap):
    """Best-fit-decreasing into LANES lanes of capacity cap. segs sorted desc."""
    buckets = [[] for _ in range(cap + 1)]
    buckets[cap] = list(range(LANES - 1, -1, -1))
    assign = [[] for _ in range(LANES)]
    for (l, b, t0) in segs:
        r = -1
        for rr in range(l, cap + 1):
            if buckets[rr]:
                r = rr
                break
        if r < 0:
            return None
        lane = buckets[r].pop()
        assign[lane].append((b, t0, l))
        buckets[r - l].append(lane)
    return assign


def _build_packing(done):
    segs = _segments(done)
    total = sum(s[0] for s in segs)
    assert total == T * B
    segs.sort(key=lambda s: -s[0])
    Tp = max(segs[0][0], (total + LANES - 1) // LANES)
    while True:
        assign = _try_pack(segs, Tp)
        if assign is not None:
            break
        Tp += 1
    src = np.full((LANES, Tp), -1, np.int64)
    mask = np.zeros((LANES, Tp), np.float32)
    for lane, lst in enumerate(assign):
        pos = 0
        for (b, t0, l) in lst:
            src[lane, pos:pos + l] = (np.arange(t0, t0 + l) * B + b)
            mask[lane, pos:pos + l - 1] = 1.0   # last slot of each segment -> 0
            pos += l
    return Tp, src, mask


def _pack_inputs(frame, reward, last_action, src, mask):
    ff = frame.reshape(T * B, SD)
    rf = reward.reshape(T * B)
    lf = last_action.reshape(T * B, AD)
    per_core = []
    for c in range(NCORES):
        s = src[c * LC:(c + 1) * LC]
        m = mask[c * LC:(c + 1) * LC]
        idx = s.T.reshape(-1)                   # tau-major columns
        ok = idx >= 0
        ii = np.where(ok, idx, 0)
        fT = np.ascontiguousarray(np.where(ok[None, :], ff[ii].T, 0.0)).astype(BF16)
        ex = np.zeros((31, idx.size), np.float32)
        ex[0] = np.where(ok, rf[ii], 0.0)
        ex[1:31] = np.where(ok[None, :], lf[ii].T, 0.0)
        mk = np.ascontiguousarray(
            np.broadcast_to(m.T.reshape(1, -1), (P, idx.size))).astype(BF16)
        per_core.append({"frameT": fT, "extrasT": ex.astype(BF16), "maskT": mk})
    return per_core


# ---------------------------------------------------------------- weights
def _gate_perm():
    perm = np.full(MW, -1, np.int64)
    pos = 0
    for j in range(4):
        for g in range(4):
            perm[pos:pos + P] = g * CORE + np.arange(j * P, (j + 1) * P)
            pos += P
    for g in range(4):                           # mixed tile: i31 _ f31 _ g31 _ o31
        perm[pos:pos + TAIL] = g * CORE + np.arange(4 * P, CORE)
        pos += TAIL
        if g < 3:
            pos += 1
    assert pos == MW
    return perm


def _colvec(v):
    out = np.zeros((P, 1), np.float32)
    v = np.asarray(v).reshape(-1)
    out[:v.size, 0] = v
    return out


def _prep_weights(p):
    perm = _gate_perm()
    ok = perm >= 0

    def reorder(wt):                             # wt [in, 4*CORE] -> [in, MW]
        out = np.zeros((wt.shape[0], MW), np.float32)
        out[:, ok] = wt[:, perm[ok]]
        return out

    w = {}
    for l in range(2):
        wi, wh = p[f"Wih{l}"], p[f"Whh{l}"]
        bias = np.zeros(MW, np.float32)
        bias[ok] = (p[f"bih{l}"] + p[f"bhh{l}"])[perm[ok]]
        wixr = reorder(wi.T)                     # [543, MW]
        whhr = reorder(wh.T)
        w[f"wix{l}"] = np.ascontiguousarray(wixr[0:512]).astype(BF16)
        w[f"whh{l}"] = np.ascontiguousarray(whhr[0:512]).astype(BF16)
        # combined tail K-tile: [x-tail 0:31 | 0 | bias@32 | 0 | h-tail 64:95 | 0]
        wct = np.zeros((P, MW), np.float32)
        wct[0:TAIL] = wixr[512:CORE]
        wct[32] = bias
        wct[64:64 + TAIL] = whhr[512:CORE]
        w[f"wct{l}"] = wct.astype(BF16)

    w["ws0t"] = np.ascontiguousarray(p["Ws0"].T).astype(BF16)       # [128,512]
    w["bs0t"] = np.ascontiguousarray(p["bs0"].reshape(4, P).T, np.float32)
    w["ws1t"] = np.ascontiguousarray(p["Ws1"].T).astype(BF16)       # [512,512]
    w["bs1t"] = np.ascontiguousarray(p["bs1"].reshape(4, P).T, np.float32)

    wp0t = np.zeros((640, 512), np.float32)
    wp0t[0:CORE] = p["Wp0"].T
    w["wp0t"] = wp0t.astype(BF16)
    w["bp0t"] = np.ascontiguousarray(p["bp0"].reshape(4, P).T, np.float32)
    w["wp1t"] = np.ascontiguousarray(p["Wp1"].T).astype(BF16)       # [512,30]
    w["bp1t"] = _colvec(p["bp1"])
    wlst = np.zeros((640, 30), np.float32)
    wlst[0:CORE] = p["Wls"].T
    w["wlst"] = wlst.astype(BF16)
    w["blst"] = _colvec(p["bls"])
    wb0t = np.zeros((640, 32), np.float32)
    wb0t[0:CORE] = p["Wb0"].T
    w["wb0t"] = wb0t.astype(BF16)
    w["bb0t"] = _colvec(p["bb0"])
    w["wb1t"] = np.ascontiguousarray(p["Wb1"].T).astype(BF16)       # [32,1]
    w["bb1t"] = _colvec(p["bb1"])
    return w


# ---------------------------------------------------------------- numpy emulator
def _emu_core(ins, p):
    """Numpy mirror of the device math (original weight order) for validation."""
    sig = lambda x: 1.0 / (1.0 + np.exp(-x))
    fT = np.asarray(ins["frameT"], np.float32)
    ex = np.asarray(ins["extrasT"], np.float32)
    mk = np.asarray(ins["maskT"], np.float32)
    TOT = fT.shape[1]
    Tp = TOT // LC
    x1 = np.maximum(p["Ws0"] @ fT + p["bs0"][:, None], 0)
    x2 = np.maximum(p["Ws1"] @ x1 + p["bs1"][:, None], 0)
    X = np.concatenate([x2, np.clip(ex[0], -1, 1)[None], ex[1:31]], 0)  # [543,TOT]
    out = np.zeros((97, TOT), np.float32)
    h1 = np.zeros((CORE, LC), np.float32)
    c1 = np.zeros_like(h1)
    h2 = np.zeros_like(h1)
    c2 = np.zeros_like(h1)
    b1 = (p["bih0"] + p["bhh0"])[:, None]
    b2 = (p["bih1"] + p["bhh1"])[:, None]
    hsw = lambda x: x * np.clip(x + 3.0, 0, 6.0) / 6.0
    for tau in range(Tp):
        sl = slice(tau * LC, (tau + 1) * LC)
        m = mk[0:1, sl]
        xs = X[:, sl]
        g = p["Wih0"] @ xs + p["Whh0"] @ h1 + b1
        i, f, gg, o = np.split(g, 4, 0)
        c1n = sig(f) * c1 + sig(i) * np.tanh(gg)
        h1r = sig(o) * np.tanh(c1n)
        h1, c1 = h1r * m, c1n * m
        g = p["Wih1"] @ h1r + p["Whh1"] @ h2 + b2
        i, f, gg, o = np.split(g, 4, 0)
        c2n = sig(f) * c2 + sig(i) * np.tanh(gg)
        y = sig(o) * np.tanh(c2n)
        h2, c2 = y * m, c2n * m
        ry = np.maximum(y, 0)
        z1 = p["Wp0"] @ ry + p["bp0"][:, None]
        am = p["Wp1"] @ hsw(z1) + p["bp1"][:, None]
        ls = np.clip(p["Wls"] @ hsw(y) + p["bls"][:, None], -20.0, 2.0)
        rb = np.maximum(p["Wb0"] @ ry + p["bb0"][:, None], 0)
        bl = p["Wb1"] @ rb + p["bb1"][:, None]
        out[0:30, sl] = am
        out[32:62, sl] = np.exp(ls)
        out[64:94, sl] = np.tanh(am)
        out[96, sl] = bl[0]
    return out


# ---------------------------------------------------------------- device kernel
def _build(Tp):
    import concourse.bass as bass
    import concourse.mybir as mybir
    from concourse import bacc
    from concourse.tile import TileContext

    f32 = mybir.dt.float32
    bf = mybir.dt.bfloat16
    AF = mybir.ActivationFunctionType
    AL = mybir.AluOpType
    n = LC
    TOT = Tp * n

    nc = bacc.Bacc(None)
    dp = lambda name, shape, dt, out=False: nc.declare_dram_parameter(
        name, list(shape), dt, isOutput=out)
    frameT = dp("frameT", (P, TOT), bf)
    extrasT = dp("extrasT", (31, TOT), bf)
    maskT = dp("maskT", (P, TOT), bf)
    ws0t = dp("ws0t", (P, 512), bf)
    bs0t = dp("bs0t", (P, 4), f32)
    ws1t = dp("ws1t", (512, 512), bf)
    bs1t = dp("bs1t", (P, 4), f32)
    wix = [dp("wix0", (512, MW), bf), dp("wix1", (512, MW), bf)]
    whh = [dp("whh0", (512, MW), bf), dp("whh1", (512, MW), bf)]
    wct = [dp("wct0", (P, MW), bf), dp("wct1", (P, MW), bf)]
    wp0t = dp("wp0t", (640, 512), bf)
    bp0t = dp("bp0t", (P, 4), f32)
    wp1t = dp("wp1t", (512, 30), bf)
    bp1t = dp("bp1t", (P, 1), f32)
    wlst = dp("wlst", (640, 30), bf)
    blst = dp("blst", (P, 1), f32)
    wb0t = dp("wb0t", (640, 32), bf)
    bb0t = dp("bb0t", (P, 1), f32)
    wb1t = dp("wb1t", (32, 1), bf)
    bb1t = dp("bb1t", (P, 1), f32)
    out97 = dp("out97", (97, TOT), f32, out=True)

    with ExitStack() as ctx:
        tc = ctx.enter_context(TileContext(nc))
        wpool = ctx.enter_context(tc.tile_pool(name="wpool", bufs=2))
        conp = ctx.enter_context(tc.tile_pool(name="conp", bufs=1))
        stp = ctx.enter_context(tc.tile_pool(name="stp", bufs=1))
        gat = ctx.enter_context(tc.tile_pool(name="gat", bufs=1))
        scr2 = ctx.enter_context(tc.tile_pool(name="scr2", bufs=2))
        scr1 = ctx.enter_context(tc.tile_pool(name="scr1", bufs=1))
        iop = ctx.enter_context(tc.tile_pool(name="iop", bufs=4))
        psp = ctx.enter_context(tc.tile_pool(name="psp", bufs=8, space="PSUM"))
        drp = ctx.enter_context(tc.tile_pool(name="drp", bufs=1, space="DRAM"))

        def dma(dst, src):
            nc.sync.dma_start(out=dst, in_=src)

        # ---- small persistent weights
        WS0 = conp.tile([P, 512], bf, tag="WS0", name="WS0")
        dma(WS0[:], ws0t[:])
        WS1 = conp.tile([P, 4 * 512], bf, tag="WS1", name="WS1")
        for k in range(4):
            dma(WS1[:, k * 512:(k + 1) * 512], ws1t[k * P:(k + 1) * P, :])
        BS0 = conp.tile([P, 4], f32, tag="BS0", name="BS0")
        dma(BS0[:], bs0t[:])
        BS1 = conp.tile([P, 4], f32, tag="BS1", name="BS1")
        dma(BS1[:], bs1t[:])
        BP0 = conp.tile([P, 4], f32, tag="BP0", name="BP0")
        dma(BP0[:], bp0t[:])
        BP1 = conp.tile([P, 1], f32, tag="BP1", name="BP1")
        dma(BP1[:], bp1t[:])
        BLS = conp.tile([P, 1], f32, tag="BLS", name="BLS")
        dma(BLS[:], blst[:])
        BB0 = conp.tile([P, 1], f32, tag="BB0", name="BB0")
        dma(BB0[:], bb0t[:])
        BB1 = conp.tile([P, 1], f32, tag="BB1", name="BB1")
        dma(BB1[:], bb1t[:])
        ZB = conp.tile([P, 1], f32, tag="ZB", name="ZB")
        nc.vector.memset(ZB[:], 0.0)
        ZBH = conp.tile([P, 1], bf, tag="ZBH", name="ZBH")
        nc.vector.memset(ZBH[:], 0.0)

        def load_big(wx, wh, wc):
            W = wpool.tile([P, 9 * MW], bf, tag="bigw", name="bigw")
            for k in range(4):
                dma(W[:, k * MW:(k + 1) * MW], wx[k * P:(k + 1) * P, :])
            for k in range(4):
                dma(W[:, (4 + k) * MW:(5 + k) * MW], wh[k * P:(k + 1) * P, :])
            dma(W[:, 8 * MW:9 * MW], wc[:])
            return W

        def new_state(nm):
            Hs = stp.tile([P, 4 * n], bf, tag="H" + nm, name="H" + nm)
            Cs = stp.tile([P, 5 * n], bf, tag="C" + nm, name="C" + nm)
            Ct = stp.tile([P, n], bf, tag="T" + nm, name="T" + nm)
            nc.vector.memset(Hs[:], 0.0)
            nc.vector.memset(Cs[:], 0.0)
            nc.vector.memset(Ct[:], 0.0)
            nc.vector.memset(Ct[32:33, :], 1.0)            # bias ones row
            return Hs, Cs, Ct

        FUNCS = [AF.Sigmoid, AF.Sigmoid, AF.Tanh, AF.Sigmoid]

        def cell(WL, xs, Hst, Cst, CTt, MK, tail_dst=None):
            """One LSTM cell step. xs(k) -> x-side rhs K-tile AP (bf16), k=0..3.
            Returns the unmasked h' tile [P, 5n] bf16 (valid rows per tile)."""
            SI = gat.tile([P, 5 * n], bf, tag="SI", name="SI")
            SF = gat.tile([P, 5 * n], bf, tag="SF", name="SF")
            TG = gat.tile([P, 5 * n], bf, tag="TG", name="TG")
            SO = gat.tile([P, 5 * n], bf, tag="SO", name="SO", bufs=2)
            GB = [SI, SF, TG, SO]

            def mseq(out_ap, mcol, mwid):
                for k in range(4):
                    nc.tensor.matmul(
                        out_ap,
                        WL[:, k * MW + mcol:k * MW + mcol + mwid],
                        xs(k), start=(k == 0), stop=False)
                for k in range(4):
                    nc.tensor.matmul(
                        out_ap,
                        WL[:, (4 + k) * MW + mcol:(4 + k) * MW + mcol + mwid],
                        Hst[:, k * n:(k + 1) * n],
                        start=False, stop=False)
                nc.tensor.matmul(
                    out_ap,
                    WL[:, 8 * MW + mcol:8 * MW + mcol + mwid],
                    CTt[:], start=False, stop=True)

            def elem(j):
                # per-hidden-tile LSTM update; pipelines under later waves' MMs
                rh = P if j < 4 else TAIL
                cs = slice(j * n, (j + 1) * n)
                nc.vector.tensor_tensor(TG[0:rh, cs], SI[0:rh, cs], TG[0:rh, cs],
                                        AL.mult)                       # sig(i)*tanh(g)
                nc.vector.tensor_tensor(SF[0:rh, cs], SF[0:rh, cs], Cst[0:rh, cs],
                                        AL.mult)                       # sig(f)*c
                nc.vector.tensor_tensor(Cst[0:rh, cs], TG[0:rh, cs], SF[0:rh, cs],
                                        AL.add)                        # c'
                nc.scalar.activation(SI[0:rh, cs], Cst[0:rh, cs], AF.Tanh,
                                     bias=ZBH[0:rh])                   # tanh(c')
                nc.vector.tensor_tensor(SO[0:rh, cs], SO[0:rh, cs], SI[0:rh, cs],
                                        AL.mult)                       # h'
                nc.vector.tensor_tensor(Cst[0:rh, cs], Cst[0:rh, cs],
                                        MK[0:rh, :], AL.mult)

            def state_write(j):
                # deferred until all this cell's h-side matmuls have read Hst
                cs = slice(j * n, (j + 1) * n)
                if j < 4:
                    nc.vector.tensor_tensor(Hst[:, cs], SO[:, cs], MK[:, :],
                                            AL.mult)
                else:
                    nc.vector.tensor_tensor(CTt[64:64 + TAIL, :],
                                            SO[0:TAIL, cs], MK[0:TAIL, :],
                                            AL.mult)

            psm = psp.tile([P, n], f32, tag="ps", name="psm")
            mseq(psm[0:127, :], 16 * P, 127)
            for g in range(4):
                nc.scalar.activation(
                    GB[g][0:TAIL, 4 * n:5 * n], psm[32 * g:32 * g + TAIL, :],
                    FUNCS[g], bias=ZB[0:TAIL])
            elem(4)
            if tail_dst is not None:
                nc.vector.tensor_copy(tail_dst[0:TAIL, :], SO[0:TAIL, 4 * n:5 * n])
            for j in range(4):
                pss = [psp.tile([P, n], f32, tag="ps", name="ps") for _ in range(4)]
                for g in range(4):
                    mseq(pss[g][:], (4 * j + g) * P, P)
                for g in range(4):
                    nc.scalar.activation(
                        GB[g][:, j * n:(j + 1) * n], pss[g][:],
                        FUNCS[g], bias=ZB[:])
                elem(j)
            for j in range(5):
                state_write(j)
            return SO

        def stage1(tau, CT1):
            FR = iop.tile([P, n], bf, tag="FR", name="FR")
            dma(FR[:], frameT[:, tau * n:(tau + 1) * n])
            X = scr2.tile([P, 4 * n], bf, tag="s5a", name="X")
            dma(CT1[0:31, :], extrasT[:, tau * n:(tau + 1) * n])
            nc.vector.tensor_scalar(CT1[0:1, :], CT1[0:1, :],
                                    1.0, -1.0, AL.min, AL.max)
            X1 = scr1.tile([P, 4 * n], bf, tag="s4a", name="X1")
            for m in range(4):
                ps = psp.tile([P, n], f32, tag="ps", name="ps1")
                nc.tensor.matmul(ps[:], WS0[:, m * P:(m + 1) * P], FR[:],
                                 start=True, stop=True)
                nc.vector.tensor_scalar(X1[:, m * n:(m + 1) * n], ps[:],
                                        BS0[:, m:m + 1], 0.0, AL.add, AL.max)
            for m in range(4):
                ps = psp.tile([P, n], f32, tag="ps", name="ps2")
                for k in range(4):
                    nc.tensor.matmul(ps[:],
                                     WS1[:, k * 512 + m * P:k * 512 + (m + 1) * P],
                                     X1[:, k * n:(k + 1) * n],
                                     start=(k == 0), stop=(k == 3))
                nc.vector.tensor_scalar(X[:, m * n:(m + 1) * n], ps[:],
                                        BS1[:, m:m + 1], 0.0, AL.add, AL.max)
            return X

        def heads(tau, Y):
            OT = iop.tile([97, n], f32, tag="OT", name="OT")
            RY = scr1.tile([P, 5 * n], bf, tag="s5b", name="RY")
            T1 = scr1.tile([P, 5 * n], bf, tag="s5d", name="T1")
            HY = scr1.tile([P, 5 * n], bf, tag="s5c", name="HY")
            for j in range(5):
                rh = P if j < 4 else TAIL
                cs = slice(j * n, (j + 1) * n)
                nc.vector.tensor_scalar_max(RY[0:rh, cs], Y[0:rh, cs], 0.0)
                nc.vector.tensor_scalar(T1[0:rh, cs], Y[0:rh, cs], 3.0, 0.0,
                                        AL.add, AL.max)
                nc.vector.tensor_scalar_min(T1[0:rh, cs], T1[0:rh, cs], 6.0)
                nc.vector.scalar_tensor_tensor(HY[0:rh, cs], Y[0:rh, cs],
                                               1.0 / 6.0, T1[0:rh, cs],
                                               AL.mult, AL.mult)
            Z1 = scr1.tile([P, 4 * n], bf, tag="s4b", name="Z1")
            T2 = scr1.tile([P, 4 * n], bf, tag="s4c", name="T2")
            for m in range(4):
                ps = psp.tile([P, n], f32, tag="ps", name="ps3")
                for k in range(5):
                    nc.tensor.matmul(ps[:],
                                     WP0[0:KH_X[k],
                                         k * 512 + m * P:k * 512 + (m + 1) * P],
                                     RY[0:KH_X[k], k * n:(k + 1) * n],
                                     start=(k == 0), stop=(k == 4))
                cs = slice(m * n, (m + 1) * n)
                nc.vector.tensor_scalar_add(Z1[:, cs], ps[:], BP0[:, m:m + 1])
                nc.vector.tensor_scalar(T2[:, cs], Z1[:, cs], 3.0, 0.0,
                                        AL.add, AL.max)
                nc.vector.tensor_scalar_min(T2[:, cs], T2[:, cs], 6.0)
                nc.vector.scalar_tensor_tensor(Z1[:, cs], Z1[:, cs], 1.0 / 6.0,
                                               T2[:, cs], AL.mult, AL.mult)
            psA = psp.tile([P, n], f32, tag="ps", name="psA")
            for k in range(4):
                nc.tensor.matmul(psA[0:30, :], WP1[:, k * 30:(k + 1) * 30],
                                 Z1[:, k * n:(k + 1) * n],
                                 start=(k == 0), stop=(k == 3))
            nc.vector.tensor_scalar_add(OT[0:30, :], psA[0:30, :], BP1[0:30, :])
            nc.scalar.activation(OT[64:94, :], OT[0:30, :], AF.Tanh,
                                 bias=ZB[0:30])
            psB = psp.tile([P, n], f32, tag="ps", name="psB")
            for k in range(5):
                nc.tensor.matmul(psB[0:32, :], WB0[0:KH_X[k], k * 32:(k + 1) * 32],
                                 RY[0:KH_X[k], k * n:(k + 1) * n],
                                 start=(k == 0), stop=(k == 4))
            RB = scr1.tile([P, n], bf, tag="s1", name="RB")
            nc.vector.tensor_scalar(RB[0:32, :], psB[0:32, :],
                                    BB0[0:32, :], 0.0, AL.add, AL.max)
            psC = psp.tile([P, n], f32, tag="ps", name="psC")
            nc.tensor.matmul(psC[0:1, :], WB1[0:32, 0:1], RB[0:32, :],
                             start=True, stop=True)
            nc.vector.tensor_scalar_add(OT[96:97, :], psC[0:1, :], BB1[0:1, :])
            psL = psp.tile([P, n], f32, tag="ps", name="psL")
            for k in range(5):
                nc.tensor.matmul(psL[0:30, :], WLS[0:KH_X[k], k * 30:(k + 1) * 30],
                                 HY[0:KH_X[k], k * n:(k + 1) * n],
                                 start=(k == 0), stop=(k == 4))
            UL = scr1.tile([P, n], f32, tag="s1b", name="UL")
            nc.vector.tensor_scalar(UL[0:30, :], psL[0:30, :],
                                    BLS[0:30, :], 2.0, AL.add, AL.min)
            nc.vector.tensor_scalar_max(UL[0:30, :], UL[0:30, :], -20.0)
            # exp(u) = sig(u)/(1-sig(u)) -- stays in the sigmoid table set
            SG = scr1.tile([P, n], f32, tag="s1c", name="SG")
            nc.scalar.activation(SG[0:30, :], UL[0:30, :], AF.Sigmoid,
                                 bias=ZB[0:30])
            nc.vector.tensor_scalar(UL[0:30, :], SG[0:30, :], -1.0, 1.0,
                                    AL.mult, AL.add)                  # 1-s
            nc.vector.reciprocal(UL[0:30, :], UL[0:30, :])
            nc.vector.tensor_tensor(OT[32:62, :], SG[0:30, :], UL[0:30, :],
                                    AL.mult)
            dma(out97[:, tau * n:(tau + 1) * n], OT[0:97, :])

        # ---- single phase: stage-1 MLP + both LSTM layers + heads per step
        H1m, C1, CT1 = new_state("1")
        H2m, C2, CT2 = new_state("2")
        MK = iop.tile([P, n], bf, tag="MK", name="MK")
        dma(MK[:], maskT[:, 0:n])
        X = stage1(0, CT1)
        WP0 = conp.tile([P, 5 * 512], bf, tag="WP0", name="WP0")
        for k in range(5):
            dma(WP0[:, k * 512:(k + 1) * 512], wp0t[k * P:(k + 1) * P, :])
        WP1 = conp.tile([P, 4 * 30], bf, tag="WP1", name="WP1")
        for k in range(4):
            dma(WP1[:, k * 30:(k + 1) * 30], wp1t[k * P:(k + 1) * P, :])
        WLS = conp.tile([P, 5 * 30], bf, tag="WLS", name="WLS")
        for k in range(5):
            dma(WLS[:, k * 30:(k + 1) * 30], wlst[k * P:(k + 1) * P, :])
        WB0 = conp.tile([P, 5 * 32], bf, tag="WB0", name="WB0")
        for k in range(5):
            dma(WB0[:, k * 32:(k + 1) * 32], wb0t[k * P:(k + 1) * P, :])
        WB1 = conp.tile([32, 1], bf, tag="WB1", name="WB1")
        dma(WB1[:], wb1t[:])
        WL0 = load_big(wix[0], whh[0], wct[0])
        WL1 = load_big(wix[1], whh[1], wct[1])
        H1R = cell(WL0, lambda k: X[:, k * n:(k + 1) * n], H1m, C1, CT1, MK,
                   tail_dst=CT2)
        # software pipeline: cell1(tau+1) fills cell2(tau)'s tail gap,
        # heads(tau) fills cell1(tau+1)'s tail gap.
        for tau in range(Tp):
            if tau + 1 < Tp:
                X = stage1(tau + 1, CT1)
            Y = cell(WL1, lambda k: H1R[:, k * n:(k + 1) * n], H2m, C2, CT2, MK)
            if tau + 1 < Tp:
                MK = iop.tile([P, n], bf, tag="MK", name="MK")
                dma(MK[:], maskT[:, (tau + 1) * n:(tau + 2) * n])
                H1R = cell(WL0, lambda k: X[:, k * n:(k + 1) * n], H1m, C1, CT1,
                           MK, tail_dst=CT2)
            heads(tau, Y)

    nc.compile()
    return nc


# ---------------------------------------------------------------- entry point
def _unpack(outs, src):
    TB = T * B
    am = np.zeros((TB, AD), np.float32)
    std = np.zeros((TB, AD), np.float32)
    act = np.zeros((TB, AD), np.float32)
    bl = np.zeros(TB, np.float32)
    for c in range(NCORES):
        s = src[c * LC:(c + 1) * LC]
        idx = s.T.reshape(-1)
        ok = idx >= 0
        o = outs[c]
        am[idx[ok]] = o[0:30][:, ok].T
        std[idx[ok]] = o[32:62][:, ok].T
        act[idx[ok]] = o[64:94][:, ok].T
        bl[idx[ok]] = o[96, ok]
    pl = np.concatenate([am, std], -1).reshape(T, B, 2 * AD)
    return pl, bl.reshape(T, B), act.reshape(T, B, AD)


def _install_ntff_hook():
    """This image's antenv lacks axon_hooks; inject it so trace=True can
    drive NTFF profiling via the libaxon .so (same recipe as trn_boot)."""
    import types
    try:
        from antenv.axon_hooks import get_axon_ntff_profile_hook  # noqa
        return
    except ImportError:
        pass
    sys.path.insert(0, "/root/.axon_site")
    from trn_agent_boot.trn_boot import _ntff_profile_via_ctypes
    hook = _ntff_profile_via_ctypes("/opt/axon/libaxon_pjrt.so")
    mod = types.ModuleType("antenv.axon_hooks")
    mod._hook = hook
    mod.set_axon_ntff_profile_hook = lambda h: setattr(mod, "_hook", h)
    mod.get_axon_ntff_profile_hook = lambda: mod._hook
    import antenv
    antenv.axon_hooks = mod
    sys.modules["antenv.axon_hooks"] = mod
    import concourse.bass_utils as bu
    bu.upload_artifacts = lambda tmpdir: f"local:{tmpdir}"


def kernel(**inputs):
    global LAST_EXEC_NS
    p = {k: np.asarray(v) for k, v in inputs.items()}
    done = np.asarray(p["done"]).astype(bool)
    Tp, src, mask = _build_packing(done)
    per_core = _pack_inputs(np.asarray(p["frame"], np.float32),
                            np.asarray(p["reward"], np.float32),
                            np.asarray(p["last_action"], np.float32),
                            src, mask)
    if os.environ.get("KMODE", "hw") == "emu":
        outs = [_emu_core(ins, p) for ins in per_core]
        return _unpack(outs, src)

    w = _prep_weights(p)
    if Tp not in _CACHE:
        _CACHE[Tp] = _build(Tp)
    nc = _CACHE[Tp]
    in_maps = [{**w, **ins} for ins in per_core]
    from concourse.bass_utils import run_bass_kernel_spmd
    trace = bool(int(os.environ.get("KTRACE", "0")))
    if trace:
        _install_ntff_hook()
    res = run_bass_kernel_spmd(
        nc, in_maps, core_ids=list(range(NCORES)),
        trace=trace)
    LAST_EXEC_NS = res.exec_time_ns
    outs = [r["out97"] for r in res.results]
    return _unpack(outs, src)
